# revision 1
# baseline (speedup 1.0000x reference)
"""Bass/Trainium2 kernel for nn_BoxFilter: 9x9 circular box-mean over
(8, 3, 1024, 1024) f32, data-parallel across 8 NeuronCores (1 image/core).

Per core, per channel, in blocks of 120 output rows:
  - vertical pass: banded matmul on PE (stationary band weights carry the
    1/81 scaling; input rows loaded with 4-row circular halo)
  - horizontal pass: one DVE tensor_tensor_scan running-box recurrence
    state[t] = state[t-1] + u[t] - u[t-9] over a wrap-padded row buffer.
"""

import numpy as np

import concourse.bacc as bacc
import concourse.mybir as mybir
import concourse.tile as tile
from concourse.bass_utils import run_bass_kernel_spmd

B, C, H, W = 8, 3, 1024, 1024
R = 4            # filter radius
WIN = 2 * R + 1  # 9
AREA = WIN * WIN
MBLK = 120       # output rows per block (input rows = MBLK + 2R = 128)
NBLK = (H + MBLK - 1) // MBLK  # 9 (last block has 64 rows)

# float32r streams 1 row/cycle on the PE (vs 4 for float32); numerics are
# validated against the fp32 reference in the block-level accuracy check.
MM_DT = mybir.dt.float32r

_CACHE: dict = {}


def _band_weights() -> np.ndarray:
    w = np.zeros((128, MBLK), dtype=np.float32)
    for m in range(MBLK):
        w[m : m + WIN, m] = 1.0 / AREA
    return w


def _build():
    f32 = mybir.dt.float32
    nc = bacc.Bacc("TRN2", target_bir_lowering=False, debug=False, num_devices=B)
    x_d = nc.dram_tensor("x", [C, H, W], MM_DT, kind="ExternalInput")
    w_d = nc.dram_tensor("w", [128, MBLK], MM_DT, kind="ExternalInput")
    o_d = nc.dram_tensor("o", [C, H, W], f32, kind="ExternalOutput")
    UW = WIN + W + 2 * R  # u buffer: [9 zeros | left wrap 4 | row 1024 | right wrap 4]
    with tile.TileContext(nc) as tc:
        with (
            tc.tile_pool(name="wpool", bufs=1) as wpool,
            tc.tile_pool(name="xpool", bufs=3) as xpool,
            tc.tile_pool(name="upool", bufs=3) as upool,
            tc.tile_pool(name="opool", bufs=3) as opool,
            tc.tile_pool(name="psum", bufs=3, space="PSUM") as psum,
        ):
            w_t = wpool.tile([128, MBLK], MM_DT)
            nc.sync.dma_start(w_t[:], w_d.ap())
            for c in range(C):
                for b in range(NBLK):
                    m = min(MBLK, H - b * MBLK)
                    k = m + 2 * R
                    r0 = b * MBLK - R  # first input row (circular)
                    x_t = xpool.tile([128, W], MM_DT, tag="x")
                    if r0 < 0:
                        nc.sync.dma_start(x_t[0 : -r0, :], x_d.ap()[c, H + r0 : H, :])
                        nc.sync.dma_start(x_t[-r0 : k, :], x_d.ap()[c, 0 : r0 + k, :])
                    elif r0 + k > H:
                        nc.sync.dma_start(x_t[0 : H - r0, :], x_d.ap()[c, r0:H, :])
                        nc.sync.dma_start(
                            x_t[H - r0 : k, :], x_d.ap()[c, 0 : r0 + k - H, :]
                        )
                    else:
                        nc.sync.dma_start(x_t[0:k, :], x_d.ap()[c, r0 : r0 + k, :])
                    v_t = psum.tile([MBLK, W], f32, tag="v")
                    for n in range(0, W, 512):
                        nc.tensor.matmul(
                            v_t[0:m, n : n + 512],
                            w_t[0:k, 0:m],
                            x_t[0:k, n : n + 512],
                            start=True,
                            stop=True,
                        )
                    u_t = upool.tile([128, UW], f32, tag="u")
                    nc.gpsimd.memset(u_t[0:m, 0:WIN], 0.0)
                    nc.scalar.copy(out=u_t[0:m, WIN : WIN + R], in_=v_t[0:m, W - R : W])
                    nc.scalar.copy(out=u_t[0:m, WIN + R : WIN + R + W], in_=v_t[0:m, :])
                    nc.scalar.copy(out=u_t[0:m, WIN + R + W : UW], in_=v_t[0:m, 0:R])
                    o_t = opool.tile([128, W + 2 * R], f32, tag="o")
                    nc.vector.tensor_tensor_scan(
                        out=o_t[0:m, :],
                        data0=u_t[0:m, WIN:UW],
                        data1=u_t[0:m, 0 : UW - WIN],
                        initial=0.0,
                        op0=mybir.AluOpType.add,
                        op1=mybir.AluOpType.subtract,
                    )
                    nc.sync.dma_start(
                        o_d.ap()[c, b * MBLK : b * MBLK + m, :],
                        o_t[0:m, 2 * R : 2 * R + W],
                    )
    nc.compile()
    return nc


def _get_nc():
    if "nc" not in _CACHE:
        _CACHE["nc"] = _build()
    return _CACHE["nc"]


def kernel(tensor: np.ndarray) -> np.ndarray:
    x = np.ascontiguousarray(np.asarray(tensor, dtype=np.float32))
    assert x.shape == (B, C, H, W), x.shape
    nc = _get_nc()
    wmat = _band_weights()
    in_maps = [{"x": x[i], "w": wmat} for i in range(B)]
    res = run_bass_kernel_spmd(nc, in_maps, core_ids=list(range(B)))
    return np.stack([res.results[i]["o"] for i in range(B)], axis=0)


# revision 3
# speedup vs baseline: 1.2382x; 1.2382x over previous
"""Bass/Trainium2 kernel for nn_BoxFilter: 9x9 circular box-mean over
(8, 3, 1024, 1024) f32, data-parallel across 8 NeuronCores (1 image/core).

Pipeline per core, per channel, in blocks of 120 output rows:
  - input arrives as bf16 hi/lo pairs (packed host-side during sharding;
    same 4 B/pixel DMA volume as fp32, fp32-accurate after PSUM accumulate)
  - vertical pass: banded ones-matmuls on PE (hi + lo accumulate in PSUM)
  - 1/81 scaling folded into the ACT PSUM->SBUF copy
  - horizontal pass: one DVE tensor_tensor_scan running-box recurrence
    state[t] = state[t-1] + u[t] - u[t-9] over a wrap-padded row buffer
  - loads issue on the Sync HWDGE ring, stores on the Scalar ring, with
    blocks paired into ~1 MB transfers.
"""

import numpy as np
import ml_dtypes

import concourse.bacc as bacc
import concourse.mybir as mybir
import concourse.tile as tile
from concourse.ap import AP
from concourse.bass_utils import run_bass_kernel_spmd

B, C, H, W = 8, 3, 1024, 1024
R = 4            # filter radius
WIN = 2 * R + 1  # 9
AREA = WIN * WIN
MBLK = 120       # output rows per block (input rows = MBLK + 2R = 128)
NBLK = (H + MBLK - 1) // MBLK  # 9 (last block has 64 rows)
UW = WIN + W + 2 * R  # u buffer: [9 zeros | left wrap 4 | row 1024 | right wrap 4]

_CACHE: dict = {}


def _band_weights() -> np.ndarray:
    w = np.zeros((128, MBLK), dtype=ml_dtypes.bfloat16)
    for m in range(MBLK):
        w[m : m + WIN, m] = 1.0
    return w


def _pack_image(x: np.ndarray) -> np.ndarray:
    """[C,H,W] f32 -> [C,H,2,W] bf16 (hi, lo) with hi+lo ~= x."""
    hi = x.astype(ml_dtypes.bfloat16)
    lo = (x - hi.astype(np.float32)).astype(ml_dtypes.bfloat16)
    return np.ascontiguousarray(np.stack([hi, lo], axis=2))


def _build():
    f32 = mybir.dt.float32
    bf16 = mybir.dt.bfloat16
    nc = bacc.Bacc("TRN2", target_bir_lowering=False, debug=False, num_devices=B)
    x_d = nc.dram_tensor("x", [C, H, 2, W], bf16, kind="ExternalInput")
    w_d = nc.dram_tensor("w", [128, MBLK], bf16, kind="ExternalInput")
    o_d = nc.dram_tensor("o", [C, H, W], f32, kind="ExternalOutput")
    # element strides in the packed input (bf16 elements)
    XROW = 2 * W              # one image row = [hi(1024) | lo(1024)]
    XCH = H * XROW            # one channel

    def vertical(v_t, x_t, w_t, m, k, q):
        """v_t[0:m, :] = banded vertical sum of block q of x_t (hi+lo)."""
        for n in range(0, W, 512):
            for s in range(2):
                nc.tensor.matmul(
                    v_t[0:m, n : n + 512],
                    w_t[0:k, 0:m],
                    x_t[0:k, q, s * W + n : s * W + n + 512],
                    start=(s == 0),
                    stop=(s == 1),
                )

    def horizontal(o_t, v_t, u_t, m, oq):
        """o_t[0:m, oq, 8:1032] = circular 9-wide running box sum of v/81."""
        nc.gpsimd.memset(u_t[0:m, 0:WIN], 0.0)
        # center copy applies the 1/81 scaling; edge wraps copy from it
        nc.scalar.mul(out=u_t[0:m, WIN + R : WIN + R + W], in_=v_t[0:m, :], mul=1.0 / AREA)
        nc.vector.tensor_copy(
            out=u_t[0:m, WIN : WIN + R],
            in_=u_t[0:m, WIN + R + W - R : WIN + R + W],
        )
        nc.vector.tensor_copy(
            out=u_t[0:m, WIN + R + W : UW], in_=u_t[0:m, WIN + R : WIN + R + R]
        )
        nc.vector.tensor_tensor_scan(
            out=o_t[0:m, oq, :],
            data0=u_t[0:m, WIN:UW],
            data1=u_t[0:m, 0 : UW - WIN],
            initial=0.0,
            op0=mybir.AluOpType.add,
            op1=mybir.AluOpType.subtract,
        )

    with tile.TileContext(nc) as tc:
        with (
            tc.tile_pool(name="wpool", bufs=1) as wpool,
            tc.tile_pool(name="xpool", bufs=3) as xpool,
            tc.tile_pool(name="upool", bufs=4) as upool,
            tc.tile_pool(name="opool", bufs=2) as opool,
            tc.tile_pool(name="psum", bufs=4, space="PSUM") as psum,
        ):
            w_t = wpool.tile([128, MBLK], bf16)
            nc.sync.dma_start(w_t[:], w_d.ap())
            for c in range(C):
                # blocks 0..7 in pairs, block 8 alone
                for j in range(4):
                    b0 = 2 * j
                    r0 = 240 * j - R
                    x_t = xpool.tile([128, 2, 2 * W], bf16, tag="x2")
                    if j == 0:
                        nc.sync.dma_start(
                            x_t[0:R, 0, :], x_d.ap()[c, H - R : H, :, :]
                        )
                        nc.sync.dma_start(
                            x_t[R:128, 0, :], x_d.ap()[c, 0 : 128 - R, :, :]
                        )
                        nc.sync.dma_start(
                            x_t[:, 1, :], x_d.ap()[c, MBLK - R : MBLK - R + 128, :, :]
                        )
                    else:
                        nc.sync.dma_start(
                            x_t[:],
                            AP(
                                x_d,
                                c * XCH + r0 * XROW,
                                [[XROW, 128], [MBLK * XROW, 2], [1, XROW]],
                            ),
                        )
                    o_t = opool.tile([MBLK, 2, W + 2 * R], f32, tag="o2")
                    for q in range(2):
                        v_t = psum.tile([MBLK, W], f32, tag="v")
                        vertical(v_t, x_t, w_t, MBLK, 128, q)
                        u_t = upool.tile([128, UW], f32, tag="u")
                        horizontal(o_t, v_t, u_t, MBLK, q)
                    nc.scalar.dma_start(
                        AP(
                            o_d,
                            (c * H + 240 * j) * W,
                            [[W, MBLK], [MBLK * W, 2], [1, W]],
                        ),
                        o_t[:, :, 2 * R : 2 * R + W],
                    )
                # block 8: 64 output rows, 72 input rows
                m, k = H - 8 * MBLK, H - 8 * MBLK + 2 * R
                r0 = 8 * MBLK - R
                x_t = xpool.tile([128, 1, 2 * W], bf16, tag="x1")
                nc.sync.dma_start(x_t[0 : H - r0, 0, :], x_d.ap()[c, r0:H, :, :])
                nc.sync.dma_start(
                    x_t[H - r0 : k, 0, :], x_d.ap()[c, 0 : k - (H - r0), :, :]
                )
                o_t = opool.tile([MBLK, 1, W + 2 * R], f32, tag="o1")
                v_t = psum.tile([MBLK, W], f32, tag="v")
                vertical(v_t, x_t, w_t, m, k, 0)
                u_t = upool.tile([128, UW], f32, tag="u")
                horizontal(o_t, v_t, u_t, m, 0)
                nc.scalar.dma_start(
                    o_d.ap()[c, 8 * MBLK : H, :], o_t[0:m, 0, 2 * R : 2 * R + W]
                )
    nc.compile()
    return nc


def _get_nc():
    if "nc" not in _CACHE:
        _CACHE["nc"] = _build()
    return _CACHE["nc"]


def _prepare_in_maps(tensor: np.ndarray) -> list:
    x = np.asarray(tensor, dtype=np.float32)
    assert x.shape == (B, C, H, W), x.shape
    wmat = _band_weights()
    return [{"x": _pack_image(x[i]), "w": wmat} for i in range(B)]


def kernel(tensor: np.ndarray) -> np.ndarray:
    nc = _get_nc()
    in_maps = _prepare_in_maps(tensor)
    res = run_bass_kernel_spmd(nc, in_maps, core_ids=list(range(B)))
    return np.stack([res.results[i]["o"] for i in range(B)], axis=0)


# revision 4
# speedup vs baseline: 1.3368x; 1.0796x over previous
"""Bass/Trainium2 kernel for nn_BoxFilter: 9x9 circular box-mean over
(8, 3, 1024, 1024) f32, data-parallel across 8 NeuronCores (1 image/core).

Pipeline per core, per channel, in blocks of 120 output rows:
  - input arrives as bf16 hi/lo pairs (packed host-side during sharding;
    same 4 B/pixel DMA volume as fp32, fp32-accurate after PSUM accumulate)
  - vertical pass: banded ones-matmuls on PE (hi + lo accumulate in PSUM)
  - 1/81 scaling folded into the ACT PSUM->SBUF copy
  - horizontal pass: one DVE tensor_tensor_scan running-box recurrence
    state[t] = state[t-1] + u[t] - u[t-9] over a wrap-padded row buffer
  - loads issue on the Sync HWDGE ring, stores on the Scalar ring, with
    blocks paired into ~1 MB transfers.
"""

import numpy as np
import ml_dtypes

import concourse.bacc as bacc
import concourse.mybir as mybir
import concourse.tile as tile
from concourse.ap import AP
from concourse.bass_utils import run_bass_kernel_spmd

B, C, H, W = 8, 3, 1024, 1024
R = 4            # filter radius
WIN = 2 * R + 1  # 9
AREA = WIN * WIN
MBLK = 120       # output rows per block (input rows = MBLK + 2R = 128)
NBLK = (H + MBLK - 1) // MBLK  # 9 (last block has 64 rows)
UW = WIN + W + 2 * R  # u buffer: [9 zeros | left wrap 4 | row 1024 | right wrap 4]

_CACHE: dict = {}


def _band_weights() -> np.ndarray:
    w = np.zeros((128, MBLK), dtype=ml_dtypes.bfloat16)
    for m in range(MBLK):
        w[m : m + WIN, m] = 1.0
    return w


def _pack_image(x: np.ndarray) -> np.ndarray:
    """[C,H,W] f32 -> [C,H,2,W] bf16 (hi, lo) with hi+lo ~= x."""
    hi = x.astype(ml_dtypes.bfloat16)
    lo = (x - hi.astype(np.float32)).astype(ml_dtypes.bfloat16)
    return np.ascontiguousarray(np.stack([hi, lo], axis=2))


def _build():
    f32 = mybir.dt.float32
    bf16 = mybir.dt.bfloat16
    nc = bacc.Bacc("TRN2", target_bir_lowering=False, debug=False, num_devices=B)
    x_d = nc.dram_tensor("x", [C, H, 2, W], bf16, kind="ExternalInput")
    w_d = nc.dram_tensor("w", [128, MBLK], bf16, kind="ExternalInput")
    o_d = nc.dram_tensor("o", [C, H, W], f32, kind="ExternalOutput")
    # element strides in the packed input (bf16 elements)
    XROW = 2 * W              # one image row = [hi(1024) | lo(1024)]
    XCH = H * XROW            # one channel

    def vertical(v_t, x_t, w_t, m, k, q):
        """v_t[0:m, :] = banded vertical sum of block q of x_t (hi+lo)."""
        for n in range(0, W, 512):
            for s in range(2):
                nc.tensor.matmul(
                    v_t[0:m, n : n + 512],
                    w_t[0:k, 0:m],
                    x_t[0:k, q, s * W + n : s * W + n + 512],
                    start=(s == 0),
                    stop=(s == 1),
                )

    def horizontal(o_t, v_t, u_t, m, oq):
        """o_t[0:m, oq, 8:1032] = circular 9-wide running box sum of v/81."""
        nc.gpsimd.memset(u_t[0:m, 0:WIN], 0.0)
        # all copies apply the 1/81 scaling while moving PSUM -> SBUF
        nc.scalar.mul(out=u_t[0:m, WIN + R : WIN + R + W], in_=v_t[0:m, :], mul=1.0 / AREA)
        nc.scalar.mul(out=u_t[0:m, WIN : WIN + R], in_=v_t[0:m, W - R : W], mul=1.0 / AREA)
        nc.scalar.mul(out=u_t[0:m, WIN + R + W : UW], in_=v_t[0:m, 0:R], mul=1.0 / AREA)
        nc.vector.tensor_tensor_scan(
            out=o_t[0:m, oq, :],
            data0=u_t[0:m, WIN:UW],
            data1=u_t[0:m, 0 : UW - WIN],
            initial=0.0,
            op0=mybir.AluOpType.add,
            op1=mybir.AluOpType.subtract,
        )

    with tile.TileContext(nc) as tc:
        with (
            tc.tile_pool(name="wpool", bufs=1) as wpool,
            tc.tile_pool(name="xpool", bufs=5) as xpool,
            tc.tile_pool(name="upool", bufs=6) as upool,
            tc.tile_pool(name="opool", bufs=4) as opool,
            tc.tile_pool(name="psum", bufs=4, space="PSUM") as psum,
        ):
            w_t = wpool.tile([128, MBLK], bf16)
            nc.sync.dma_start(w_t[:], w_d.ap())
            for c in range(C):
                # blocks 0..7 in pairs, block 8 alone
                for j in range(4):
                    b0 = 2 * j
                    r0 = 240 * j - R
                    x_t = xpool.tile([128, 2, 2 * W], bf16, tag="x2")
                    if j == 0:
                        nc.sync.dma_start(
                            x_t[0:R, 0, :], x_d.ap()[c, H - R : H, :, :]
                        )
                        nc.sync.dma_start(
                            x_t[R:128, 0, :], x_d.ap()[c, 0 : 128 - R, :, :]
                        )
                        nc.sync.dma_start(
                            x_t[:, 1, :], x_d.ap()[c, MBLK - R : MBLK - R + 128, :, :]
                        )
                    else:
                        nc.sync.dma_start(
                            x_t[:],
                            AP(
                                x_d,
                                c * XCH + r0 * XROW,
                                [[XROW, 128], [MBLK * XROW, 2], [1, XROW]],
                            ),
                        )
                    o_t = opool.tile([MBLK, 2, W + 2 * R], f32, tag="o2")
                    for q in range(2):
                        v_t = psum.tile([MBLK, W], f32, tag="v")
                        vertical(v_t, x_t, w_t, MBLK, 128, q)
                        u_t = upool.tile([128, UW], f32, tag="u")
                        horizontal(o_t, v_t, u_t, MBLK, q)
                    store_eng = nc.scalar if (c * 4 + j) % 2 == 0 else nc.sync
                    store_eng.dma_start(
                        AP(
                            o_d,
                            (c * H + 240 * j) * W,
                            [[W, MBLK], [MBLK * W, 2], [1, W]],
                        ),
                        o_t[:, :, 2 * R : 2 * R + W],
                    )
                # block 8: 64 output rows, 72 input rows
                m, k = H - 8 * MBLK, H - 8 * MBLK + 2 * R
                r0 = 8 * MBLK - R
                x_t = xpool.tile([128, 1, 2 * W], bf16, tag="x1")
                nc.sync.dma_start(x_t[0 : H - r0, 0, :], x_d.ap()[c, r0:H, :, :])
                nc.sync.dma_start(
                    x_t[H - r0 : k, 0, :], x_d.ap()[c, 0 : k - (H - r0), :, :]
                )
                o_t = opool.tile([MBLK, 1, W + 2 * R], f32, tag="o1")
                v_t = psum.tile([MBLK, W], f32, tag="v")
                vertical(v_t, x_t, w_t, m, k, 0)
                u_t = upool.tile([128, UW], f32, tag="u")
                horizontal(o_t, v_t, u_t, m, 0)
                nc.scalar.dma_start(
                    o_d.ap()[c, 8 * MBLK : H, :], o_t[0:m, 0, 2 * R : 2 * R + W]
                )
    nc.compile()
    return nc


def _get_nc():
    if "nc" not in _CACHE:
        _CACHE["nc"] = _build()
    return _CACHE["nc"]


def _prepare_in_maps(tensor: np.ndarray) -> list:
    x = np.asarray(tensor, dtype=np.float32)
    assert x.shape == (B, C, H, W), x.shape
    wmat = _band_weights()
    return [{"x": _pack_image(x[i]), "w": wmat} for i in range(B)]


def kernel(tensor: np.ndarray) -> np.ndarray:
    nc = _get_nc()
    in_maps = _prepare_in_maps(tensor)
    res = run_bass_kernel_spmd(nc, in_maps, core_ids=list(range(B)))
    return np.stack([res.results[i]["o"] for i in range(B)], axis=0)


# revision 5
# speedup vs baseline: 1.3487x; 1.0089x over previous
"""Bass/Trainium2 kernel for nn_BoxFilter: 9x9 circular box-mean over
(8, 3, 1024, 1024) f32, data-parallel across 8 NeuronCores (1 image/core).

Pipeline per core, per channel, in blocks of 120 output rows:
  - input arrives as bf16 hi/lo pairs (packed host-side during sharding;
    same 4 B/pixel DMA volume as fp32, fp32-accurate after PSUM accumulate)
  - vertical pass: banded ones-matmuls on PE (hi + lo accumulate in PSUM)
  - 1/81 scaling folded into the ACT PSUM->SBUF copy
  - horizontal pass: one DVE tensor_tensor_scan running-box recurrence
    state[t] = state[t-1] + u[t] - u[t-9] over a wrap-padded row buffer
  - loads issue on the Sync HWDGE ring, stores on the Scalar ring, with
    blocks paired into ~1 MB transfers.
"""

import numpy as np
import ml_dtypes

import concourse.bacc as bacc
import concourse.mybir as mybir
import concourse.tile as tile
from concourse.ap import AP
from concourse.bass_utils import run_bass_kernel_spmd

B, C, H, W = 8, 3, 1024, 1024
R = 4            # filter radius
WIN = 2 * R + 1  # 9
AREA = WIN * WIN
MBLK = 120       # output rows per block (input rows = MBLK + 2R = 128)
NBLK = (H + MBLK - 1) // MBLK  # 9 (last block has 64 rows)
UW = WIN + W + 2 * R  # u buffer: [9 zeros | left wrap 4 | row 1024 | right wrap 4]

_CACHE: dict = {}


def _band_weights() -> np.ndarray:
    w = np.zeros((128, MBLK), dtype=ml_dtypes.bfloat16)
    for m in range(MBLK):
        w[m : m + WIN, m] = 1.0
    return w


def _pack_image(x: np.ndarray) -> np.ndarray:
    """[C,H,W] f32 -> [C,H,2,W] bf16 (hi, lo) with hi+lo ~= x."""
    hi = x.astype(ml_dtypes.bfloat16)
    lo = (x - hi.astype(np.float32)).astype(ml_dtypes.bfloat16)
    return np.ascontiguousarray(np.stack([hi, lo], axis=2))


def _build():
    f32 = mybir.dt.float32
    bf16 = mybir.dt.bfloat16
    nc = bacc.Bacc("TRN2", target_bir_lowering=False, debug=False, num_devices=B)
    x_d = nc.dram_tensor("x", [C, H, 2, W], bf16, kind="ExternalInput")
    w_d = nc.dram_tensor("w", [128, MBLK], bf16, kind="ExternalInput")
    o_d = nc.dram_tensor("o", [C, H, W], f32, kind="ExternalOutput")
    # element strides in the packed input (bf16 elements)
    XROW = 2 * W              # one image row = [hi(1024) | lo(1024)]
    XCH = H * XROW            # one channel

    def vertical(v_t, x_t, w_t, m, k, q):
        """v_t[0:m, :] = banded vertical sum of block q of x_t (hi+lo)."""
        for n in range(0, W, 512):
            for s in range(2):
                nc.tensor.matmul(
                    v_t[0:m, n : n + 512],
                    w_t[0:k, 0:m],
                    x_t[0:k, q, s * W + n : s * W + n + 512],
                    start=(s == 0),
                    stop=(s == 1),
                )

    def horizontal(o_t, v_t, u_t, m, oq):
        """o_t[0:m, oq, 8:1032] = circular 9-wide running box sum of v/81."""
        nc.gpsimd.memset(u_t[0:m, 0:WIN], 0.0)
        # all copies apply the 1/81 scaling while moving PSUM -> SBUF
        nc.scalar.mul(out=u_t[0:m, WIN + R : WIN + R + W], in_=v_t[0:m, :], mul=1.0 / AREA)
        nc.scalar.mul(out=u_t[0:m, WIN : WIN + R], in_=v_t[0:m, W - R : W], mul=1.0 / AREA)
        nc.scalar.mul(out=u_t[0:m, WIN + R + W : UW], in_=v_t[0:m, 0:R], mul=1.0 / AREA)
        nc.vector.tensor_tensor_scan(
            out=o_t[0:m, oq, :],
            data0=u_t[0:m, WIN:UW],
            data1=u_t[0:m, 0 : UW - WIN],
            initial=0.0,
            op0=mybir.AluOpType.add,
            op1=mybir.AluOpType.subtract,
        )

    with tile.TileContext(nc) as tc:
        with (
            tc.tile_pool(name="wpool", bufs=1) as wpool,
            tc.tile_pool(name="xpool", bufs=7) as xpool,
            tc.tile_pool(name="upool", bufs=8) as upool,
            tc.tile_pool(name="opool", bufs=6) as opool,
            tc.tile_pool(name="psum", bufs=4, space="PSUM") as psum,
        ):
            w_t = wpool.tile([128, MBLK], bf16)
            nc.sync.dma_start(w_t[:], w_d.ap())
            for c in range(C):
                # block 8 first: its small load primes the pipeline
                m, k = H - 8 * MBLK, H - 8 * MBLK + 2 * R
                r0 = 8 * MBLK - R
                x_t = xpool.tile([128, 1, 2 * W], bf16, tag="x1")
                eng8 = nc.scalar if c == 0 else nc.sync
                eng8.dma_start(x_t[0 : H - r0, 0, :], x_d.ap()[c, r0:H, :, :])
                eng8.dma_start(
                    x_t[H - r0 : k, 0, :], x_d.ap()[c, 0 : k - (H - r0), :, :]
                )
                o_t = opool.tile([MBLK, 1, W + 2 * R], f32, tag="o1")
                v_t = psum.tile([MBLK, W], f32, tag="v")
                vertical(v_t, x_t, w_t, m, k, 0)
                u_t = upool.tile([128, UW], f32, tag="u")
                horizontal(o_t, v_t, u_t, m, 0)
                nc.scalar.dma_start(
                    o_d.ap()[c, 8 * MBLK : H, :], o_t[0:m, 0, 2 * R : 2 * R + W]
                )
                # blocks 0..7 in pairs
                for j in range(4):
                    r0 = 240 * j - R
                    x_t = xpool.tile([128, 2, 2 * W], bf16, tag="x2")
                    if j == 0:
                        nc.sync.dma_start(
                            x_t[0:R, 0, :], x_d.ap()[c, H - R : H, :, :]
                        )
                        nc.sync.dma_start(
                            x_t[R:64, 0, :], x_d.ap()[c, 0 : 64 - R, :, :]
                        )
                        nc.scalar.dma_start(
                            x_t[64:128, 0, :], x_d.ap()[c, 64 - R : 128 - R, :, :]
                        )
                        nc.sync.dma_start(
                            x_t[0:64, 1, :], x_d.ap()[c, MBLK - R : MBLK - R + 64, :, :]
                        )
                        nc.scalar.dma_start(
                            x_t[64:128, 1, :],
                            x_d.ap()[c, MBLK - R + 64 : MBLK - R + 128, :, :],
                        )
                    else:
                        nc.sync.dma_start(
                            x_t[:],
                            AP(
                                x_d,
                                c * XCH + r0 * XROW,
                                [[XROW, 128], [MBLK * XROW, 2], [1, XROW]],
                            ),
                        )
                    o_t = opool.tile([MBLK, 2, W + 2 * R], f32, tag="o2")
                    for q in range(2):
                        v_t = psum.tile([MBLK, W], f32, tag="v")
                        vertical(v_t, x_t, w_t, MBLK, 128, q)
                        u_t = upool.tile([128, UW], f32, tag="u")
                        horizontal(o_t, v_t, u_t, MBLK, q)
                    store_eng = nc.scalar if (c * 4 + j) % 2 == 0 else nc.sync
                    store_eng.dma_start(
                        AP(
                            o_d,
                            (c * H + 240 * j) * W,
                            [[W, MBLK], [MBLK * W, 2], [1, W]],
                        ),
                        o_t[:, :, 2 * R : 2 * R + W],
                    )
    nc.compile()
    return nc


def _get_nc():
    if "nc" not in _CACHE:
        _CACHE["nc"] = _build()
    return _CACHE["nc"]


def _prepare_in_maps(tensor: np.ndarray) -> list:
    x = np.asarray(tensor, dtype=np.float32)
    assert x.shape == (B, C, H, W), x.shape
    wmat = _band_weights()
    return [{"x": _pack_image(x[i]), "w": wmat} for i in range(B)]


def kernel(tensor: np.ndarray) -> np.ndarray:
    nc = _get_nc()
    in_maps = _prepare_in_maps(tensor)
    res = run_bass_kernel_spmd(nc, in_maps, core_ids=list(range(B)))
    return np.stack([res.results[i]["o"] for i in range(B)], axis=0)
